# revision 43
# baseline (speedup 1.0000x reference)
"""Multi-head attention (B=2, N=2048, C=1024, H=16) on 8 TRN2 NeuronCores.

Sharding: tensor-parallel over heads (2 heads/core) for qkv+attention,
then AllToAll to token-shard the output projection.

v2: one globally software-pipelined schedule built around saturating the
Scalar (activation) engine, which has ~142us of exp work - the hard
bottleneck of this shard layout:
  - attention iterations (sst pair -> exp -> hav pair) start as soon as
    K(0)/Q(0)/V(0..3) exist (~8us), not after the full qkv phase
  - all remaining qkv matmuls (own batch, next batch) and the output
    projections are interleaved into the iteration stream as PE filler,
    budgeted so the PE always has work (keeps its p-state ramped at
    2.4GHz) but never starves ScalarE of sst inputs
  - sst for iteration i+1 is emitted before hav for iteration i so the
    PE never waits on exp latency
  - reciprocal_approx_fast instead of reciprocal (5x) for the softmax
    denominators; normalize chain kept off the critical path
  - per-qb scatter into the AllToAll buffer and per-group gather out of
    it are single strided DMAs (transposed APs) instead of 8-16 small
    serialized DMAs
  - PE warmup matmuls at t=0 and across the final AllToAll wait keep the
    p-state up where there is no real work to overlap
"""

import numpy as np
import ml_dtypes

import concourse.mybir as mybir
import concourse.tile as tile
from concourse import bacc
from concourse.bass_utils import run_bass_kernel_spmd

F32 = mybir.dt.float32
BF16 = mybir.dt.bfloat16
EXP = mybir.ActivationFunctionType.Exp

N_CORES = 8
B = 2
C = 1024
H = 16
D = 64
HPC = H // N_CORES          # heads per core
CH = HPC * D                # channels per core (128)
KT_C = C // 128             # contraction tiles (8)

RELEASE_FIRST = 34          # iters before first collective's proj releases
RELEASE_REST = 22           # iters for later batch-0 collectives
FILL_PER_ITER = 2.35        # filler matmuls per attention iteration
HAV_LAG = 8                 # max iterations hav may trail sst/exp


def groups_for(b, QB):
    g = [list(range(i, min(i + 2, QB))) for i in range(0, QB, 2)]
    if b == B - 1 and len(g) > 1:
        g = g[:-1] + [[q] for q in g[-1]]
    return g


def build_program(N=2048, n_cores=N_CORES, mm_dt=BF16):
    assert N % 512 == 0
    QB = N // 512            # 512-wide query/column blocks per batch
    NK = N // 128            # 128-row key tiles per batch
    scale = float(D) ** -0.5
    TOKB = N // n_cores

    nc = bacc.Bacc("TRN2", target_bir_lowering=False, debug=False,
                   num_devices=n_cores)

    # x host-packed per 512-column block: xR[b, cb, r, k*512+c]
    xR_d = nc.dram_tensor("xR", [B, QB, 128, KT_C * 512], mm_dt,
                          kind="ExternalInput")
    wqT_d = nc.dram_tensor("wqT", [128, KT_C * CH], mm_dt, kind="ExternalInput")
    wkT_d = nc.dram_tensor("wkT", [128, KT_C * CH], mm_dt, kind="ExternalInput")
    wvT_d = nc.dram_tensor("wvT", [128, KT_C * CH], mm_dt, kind="ExternalInput")
    pwT_d = nc.dram_tensor("pwT", [128, KT_C * C], mm_dt, kind="ExternalInput")
    onesb_d = nc.dram_tensor("onesb", [1, 128], mm_dt, kind="ExternalInput")
    pbb_d = nc.dram_tensor("pbb", [C], mm_dt, kind="ExternalInput")
    out_d = nc.dram_tensor("out", [B, TOKB, C], F32, kind="ExternalOutput")

    lp = nc.allow_low_precision("bf16 matmul pipeline")

    with tile.TileContext(nc) as tc:
        with (tc.tile_pool(name="sb", bufs=1) as sb,
              tc.tile_pool(name="ps", bufs=1, space="PSUM") as ps,
              tc.tile_pool(name="dr", bufs=1, space="DRAM") as dr,
              lp):
            # PSUM budget (8 banks): sst 2x2 + hav0 + hav1 + acc 2 (shared
            # by qkv chains, projections, warmup)

            # ---- weights / constants; DMA order: wk then x(0,0) first so
            # the first K chain can start ~4us in ----
            wkt = sb.tile([128, KT_C * CH], mm_dt, tag="wk", bufs=1, name="wk")
            nc.sync.dma_start(wkt[:], wkT_d.ap())

            xt = {}

            def load_x(b, cb, split=False):
                t = sb.tile([128, KT_C * 512], mm_dt, tag="xt", bufs=2 * QB,
                            name=f"x{b}_{cb}")
                if split:
                    # halves (k-major) so K0's first matmuls start earlier
                    half = KT_C * 256
                    nc.sync.dma_start(t[:, 0:half], xR_d.ap()[b, cb][:, 0:half])
                    nc.sync.dma_start(t[:, half:], xR_d.ap()[b, cb][:, half:])
                else:
                    nc.sync.dma_start(t[:], xR_d.ap()[b, cb])
                xt[(b, cb)] = t

            load_x(0, 0, split=True)
            wqt = sb.tile([128, KT_C * CH], mm_dt, tag="wq", bufs=1, name="wq")
            nc.sync.dma_start(wqt[:], wqT_d.ap())
            wvt = sb.tile([128, KT_C * CH], mm_dt, tag="wv", bufs=1, name="wv")
            nc.sync.dma_start(wvt[:], wvT_d.ap())
            onesb = sb.tile([1, 128], mm_dt, tag="onesb", bufs=1)
            nc.sync.dma_start(onesb[:], onesb_d.ap())
            pbb_sb = sb.tile([1, C], mm_dt, tag="pbb", bufs=1)
            nc.sync.dma_start(pbb_sb[:], pbb_d.ap().unsqueeze(0))
            # vau ones-stripes via gpsimd memset (a strided DMA for these
            # costs ~4us of sync-queue time each and delays the x loads)
            vau = [[] for _ in range(B)]
            for vb in range(B):
                for h in range(2):
                    t = sb.tile([128, 65 * NK], mm_dt, tag=f"vau{h}", bufs=2,
                                name=f"vau{vb}_{h}")
                    nc.gpsimd.memset(t[:, 64::65], 1.0)
                    vau[vb].append(t)
            for cb in range(1, QB):
                load_x(0, cb)
            for cb in range(QB):
                load_x(1, cb)
            pwt = sb.tile([128, KT_C * C], mm_dt, tag="pw", bufs=1, name="pw")
            nc.sync.dma_start(pwt[:], pwT_d.ap())

            wk = [wkt[:, CH * k:CH * k + CH] for k in range(KT_C)]
            wq = [wqt[:, CH * k:CH * k + CH] for k in range(KT_C)]
            wv = [wvt[:, CH * k:CH * k + CH] for k in range(KT_C)]
            pw = [pwt[:, C * k:C * k + C] for k in range(KT_C)]

            # ---- PE warmup: ramp the p-state before real work lands ----
            def warmup(n):
                wacc = ps.tile([128, 512], F32, tag="acc", bufs=2,
                               name="warm")
                for i in range(n):
                    nc.tensor.matmul(wacc[:], wkt[:, 0:128], wkt[:, 0:512],
                                     start=(i == 0), stop=(i == n - 1))

            warmup(4)

            # ---- per-batch qkv generators with milestone tracking ----
            kts = [{} for _ in range(B)]
            qts = [{} for _ in range(B)]
            done = [set() for _ in range(B)]

            def qkv_gen(b):
                def xs(cb, k, lo, w):
                    return xt[(b, cb)][:, 512 * k + lo:512 * k + lo + w]

                def kq_chain(which, qb, wlist, store, tag):
                    acc = ps.tile([128, 512], F32, tag="acc", bufs=2,
                                  name=f"{which}acc{b}_{qb}")
                    for k in range(KT_C):
                        nc.tensor.matmul(acc[:], wlist[k], xs(qb, k, 0, 512),
                                         start=(k == 0), stop=(k == KT_C - 1))
                        yield 1
                    tgt = sb.tile([128, 512], mm_dt, tag=tag, bufs=2 * QB + 1,
                                  name=f"{tag}{b}_{qb}")
                    nc.vector.tensor_copy(tgt[:], acc[:])
                    store[qb] = tgt
                    done[b].add((which, qb))

                def k0_chunk(lo, w, kt_t, mark):
                    # first K chain in two column chunks: the first sst tile
                    # only needs keys 0:128, so exp can start ~5us earlier
                    acc = ps.tile([128, w], F32, tag="acc", bufs=2,
                                  name=f"k0acc{b}_{lo}")
                    for k in range(KT_C):
                        nc.tensor.matmul(acc[:], wk[k], xs(0, k, lo, w),
                                         start=(k == 0), stop=(k == KT_C - 1))
                        yield 1
                    nc.vector.tensor_copy(kt_t[:, lo:lo + w], acc[:])
                    done[b].add(mark)

                def v_chain(tt):
                    cb, off = tt // 4, (tt % 4) * 128
                    acc = ps.tile([128, 128], F32, tag="acc", bufs=2,
                                  name=f"vacc{b}_{tt}")
                    for k in range(KT_C):
                        nc.tensor.matmul(acc[:], xs(cb, k, off, 128), wv[k],
                                         start=(k == 0), stop=(k == KT_C - 1))
                        yield 1
                    nc.vector.tensor_copy(
                        vau[b][0][:, 65 * tt:65 * tt + 64], acc[:, 0:64])
                    nc.vector.tensor_copy(
                        vau[b][1][:, 65 * tt:65 * tt + 64], acc[:, 64:128])
                    done[b].add(('V', tt))

                yield from kq_chain('K', 0, wk, kts[b], "kt")
                yield from kq_chain('Q', 0, wq, qts[b], "qt")
                for tt in range(4):
                    yield from v_chain(tt)
                for cb in range(1, QB):
                    yield from kq_chain('K', cb, wk, kts[b], "kt")
                    if cb == 1 and QB > 1:
                        yield from kq_chain('Q', 1, wq, qts[b], "qt")
                    for tt in range(4 * cb, 4 * cb + 4):
                        yield from v_chain(tt)
                for qb in range(2, QB):
                    yield from kq_chain('Q', qb, wq, qts[b], "qt")

            gens = [qkv_gen(b) for b in range(B)]

            units = [0]          # PE matmuls emitted this iteration

            def force(b, key):
                while key not in done[b]:
                    if next(gens[b], None) is None:
                        if key in done[b]:
                            break
                        raise RuntimeError(f"gen {b} dry before {key}")
                    units[0] += 1

            # ---- filler pool (priority order) + proj release queue ----
            def dummy_gen(n):
                # p-state keep-alive when real filler runs dry
                dacc = ps.tile([128, 512], F32, tag="acc", bufs=2,
                               name="dummy")
                for i in range(n):
                    nc.tensor.matmul(dacc[:], wkt[:, 0:128], wkt[:, 0:512],
                                     start=(i == 0), stop=(i == n - 1))
                    yield 1

            fillq = list(gens) + [dummy_gen(60)]
            proj_ripe = []       # (release_at_iter, generator)
            it_counter = [0]

            def fill_pull():
                while fillq:
                    if next(fillq[0], None) is None:
                        fillq.pop(0)
                        continue
                    return True
                return False

            # ---- projection job per (batch, group) ----
            def proj_job(b, a2a_out, tokg, tok0, gi):
                pl = sb.tile([128, KT_C * tokg], mm_dt, tag="pl", bufs=3,
                             name=f"pl{b}_{gi}")
                nc.sync.dma_start(pl.rearrange("p (k t) -> p k t", k=KT_C),
                                  a2a_out.transpose([1, 0, 2]))
                yield 0
                for oh in range(2):
                    os_ = slice(512 * oh, 512 * oh + 512)
                    acc = ps.tile([128, 512], F32, tag="acc", bufs=2,
                                  name=f"pacc{b}_{gi}_{oh}")
                    for k in range(KT_C):
                        nc.tensor.matmul(
                            acc[0:tokg, :], pl[:, tokg * k:tokg * k + tokg],
                            pw[k][:, os_], start=(k == 0), stop=False)
                        yield 1
                    nc.tensor.matmul(acc[0:tokg, :], onesb[0:1, 0:tokg],
                                     pbb_sb[0:1, os_], start=False, stop=True)
                    yield 1
                    osb = sb.tile([128, 512], F32, tag="osb", bufs=2,
                                  name=f"osb{b}_{gi}_{oh}")
                    nc.vector.tensor_copy(osb[0:tokg, :], acc[0:tokg, :])
                    nc.sync.dma_start(
                        out_d.ap()[b, tok0:tok0 + tokg, os_], osb[0:tokg, :])

            # ---- attention building blocks ----
            hav = {}             # h -> current psum tile
            n_coll = [0]

            def emit_sst_exp(b, qb, t):
                force(b, ('K', t // 4))
                force(b, ('Q', qb))
                sst = ps.tile([128, 1024], F32, tag="sst", bufs=2,
                              name=f"sst{b}_{qb}_{t}")
                pt = sb.tile([128, 1024], mm_dt, tag="pt", bufs=14,
                             name=f"pt{b}_{qb}_{t}")
                kb, ko = t // 4, t % 4
                ks = slice(128 * ko, 128 * ko + 128)
                for h in range(2):
                    hs = slice(64 * h, 64 * h + 64)
                    nc.tensor.matmul(sst[:, 512 * h:512 * h + 512],
                                     kts[b][kb][hs, ks], qts[b][qb][hs, :],
                                     start=True, stop=True)
                units[0] += 1      # pair runs concurrently: one PE slot
                nc.scalar.activation(pt[:], sst[:], EXP, scale=scale)
                return pt

            grp_state = {}       # (b, gi) -> dict with a2a_in etc.

            def emit_hav(b, qb, t, pt):
                force(b, ('V', t))
                if t == 0:
                    for h in range(2):
                        hav[h] = ps.tile([65, 512], F32, tag=f"hav{h}",
                                         bufs=1, name=f"hav{b}_{qb}_{h}")
                for h in range(2):
                    nc.tensor.matmul(hav[h][:],
                                     vau[b][h][:, 65 * t:65 * t + 65],
                                     pt[:, 512 * h:512 * h + 512],
                                     start=(t == 0), stop=(t == NK - 1))
                units[0] += 2
                if t == NK - 1:
                    finish_qb(b, qb)

            def finish_qb(b, qb):
                grps = groups_for(b, QB)
                gi = next(i for i, g in enumerate(grps) if qb in g)
                grp = grps[gi]
                p = grp.index(qb)
                tokg = 512 * len(grp) // n_cores
                npr = 512 // tokg
                st = grp_state.setdefault((b, gi), {})
                if "a2a_in" not in st:
                    st["a2a_in"] = dr.tile([n_cores, CH, tokg], mm_dt,
                                           tag="a2ain", bufs=5,
                                           name=f"a2ain{b}_{gi}")
                # normalize: h/denom with approx reciprocal, then one
                # strided scatter DMA into the group's AllToAll buffer
                ht = sb.tile([128, 512], mm_dt, tag="ht", bufs=2,
                             name=f"ht{b}_{qb}")
                for h in range(2):
                    # one copy frees the hav PSUM bank for the next qb as
                    # early as possible (the next block's matmuls wait on it)
                    hcp = sb.tile([65, 512], F32, tag=f"hcp{h}", bufs=2,
                                  name=f"hcp{b}_{qb}_{h}")
                    nc.vector.tensor_copy(hcp[:], hav[h][:])
                    # denominator row to a partition-0 tile: the custom-DVE
                    # approx reciprocal mishandles base_partition != 0
                    dnm = sb.tile([1, 512], F32, tag=f"dnm{h}", bufs=2,
                                  name=f"dnm{b}_{qb}_{h}")
                    nc.vector.tensor_copy(dnm[:], hcp[64:65, :])
                    nrr = sb.tile([1, 512], F32, tag=f"nrr{h}", bufs=2,
                                  name=f"nrr{b}_{qb}_{h}")
                    nc.vector.reciprocal_approx_fast(nrr[:], dnm[:])
                    bcs = sb.tile([64, 512], F32, tag=f"bcs{h}", bufs=2,
                                  name=f"bcs{b}_{qb}_{h}")
                    nc.gpsimd.partition_broadcast(bcs[:], nrr[:])
                    nc.vector.tensor_mul(ht[64 * h:64 * h + 64, :],
                                         hcp[0:64, :], bcs[:])
                a2a_in = st["a2a_in"]
                nc.sync.dma_start(
                    a2a_in[p * npr:(p + 1) * npr].transpose([1, 0, 2]),
                    ht.rearrange("p (s t) -> p s t", s=npr))
                if p == len(grp) - 1:
                    a2a_out = dr.tile([n_cores, CH, tokg], mm_dt,
                                      tag="a2aout", bufs=5,
                                      name=f"a2aout{b}_{gi}")
                    nc.gpsimd.collective_compute(
                        "AllToAll", mybir.AluOpType.bypass,
                        replica_groups=[list(range(n_cores))],
                        ins=[a2a_in.opt()], outs=[a2a_out.opt()])
                    tok0 = sum(512 * len(g) // n_cores for g in grps[:gi])
                    # last batch's projs run in the drain, where they double
                    # as PE filler across the final collectives' latency
                    if b == B - 1:
                        delay = 10 ** 9
                    elif n_coll[0] == 0:
                        delay = RELEASE_FIRST
                    else:
                        delay = RELEASE_REST
                    n_coll[0] += 1
                    proj_ripe.append((it_counter[0] + delay,
                                      proj_job(b, a2a_out, tokg, tok0, gi)))

            # ---- the global iteration stream ----
            iters = [(b, qb, t) for b in range(B) for qb in range(QB)
                     for t in range(NK)]
            pending = []
            credit = 0.0
            for (b, qb, t) in iters:
                units[0] = 0
                pt = emit_sst_exp(b, qb, t)
                pending.append((b, qb, t, pt))
                # emit hav once its V tile exists, or when the lag cap hits;
                # deferring spreads the V-chain crunch past the first qb
                while pending and (len(pending) > HAV_LAG
                                   or ('V', pending[0][2]) in done[pending[0][0]]):
                    emit_hav(*pending.pop(0))
                # release ripe proj jobs into the filler pool, ahead of the
                # dummy keep-alive generator at the tail of the queue
                for ent in list(proj_ripe):
                    if ent[0] <= it_counter[0]:
                        fillq.insert(max(0, len(fillq) - 1), ent[1])
                        proj_ripe.remove(ent)
                # budget: attention fixed cost ~3 slots vs exp cadence ~5.2
                credit += FILL_PER_ITER + 3.0 - units[0]
                credit = max(-6.0, min(8.0, credit))
                while credit >= 1.0 and fill_pull():
                    credit -= 1.0
                it_counter[0] += 1

            # ---- drain: final havs + chain, remaining filler, tail projs ----
            # prime the pl loads of already-collected groups first, so they
            # are not head-of-line blocked behind the final scatter on the
            # sync queue (the last group's own pl must NOT be primed here:
            # its AllToAll depends on the scatter emitted below)
            proj_ripe.sort(key=lambda e: e[0])
            primed = [g for _, g in proj_ripe]
            proj_ripe.clear()
            for g in primed:
                next(g, None)
            while pending:
                emit_hav(*pending.pop(0))
            while fill_pull():
                pass
            jobs = primed + [g for _, g in proj_ripe]
            proj_ripe.clear()
            for g in jobs[:-1]:
                while next(g, None) is not None:
                    pass
            # keep the PE ramped across the last AllToAll's latency
            warmup(40)
            for g in jobs[-1:]:
                while next(g, None) is not None:
                    pass

    nc.compile()
    return nc


def shard_inputs(x, qkv_w, proj_w, proj_b, n_cores=N_CORES, mm_dt=BF16):
    """Host-side sharding: pre-pack activations/weights, slice heads."""
    npdt = ml_dtypes.bfloat16 if mm_dt == BF16 else np.float32
    x = np.asarray(x)
    Bx, N, Cx = x.shape
    QB = N // 512
    xT = np.ascontiguousarray(np.transpose(x, (0, 2, 1))).astype(npdt)
    # xR[b, cb, r, k*512+c] = xT[b, 128k+r, 512cb+c]
    xR = np.ascontiguousarray(
        xT.reshape(Bx, KT_C, 128, QB, 512).transpose(0, 3, 2, 1, 4)
        .reshape(Bx, QB, 128, KT_C * 512))
    qkv_w = np.asarray(qkv_w)

    def pack(wT):  # [C, cols] -> [128, (C//128)*cols], k-tiles side by side
        cdim, cols = wT.shape
        return np.ascontiguousarray(
            wT.reshape(cdim // 128, 128, cols).transpose(1, 0, 2)
            .reshape(128, -1)).astype(npdt)

    pwT = pack(np.asarray(proj_w).T)
    pb = np.ascontiguousarray(np.asarray(proj_b)).astype(np.float32)
    in_maps = []
    for i in range(n_cores):
        cs = slice(CH * i, CH * i + CH)
        in_maps.append({
            "xR": xR,
            "wqT": pack(qkv_w[cs, :].T),
            "wkT": pack(qkv_w[C:][cs, :].T),
            "wvT": pack(qkv_w[2 * C:][cs, :].T),
            "pwT": pwT,
            "onesb": np.ones((1, 128), dtype=npdt),
            "pbb": pb.astype(npdt),
        })
    return in_maps


def assemble_output(res, N, n_cores=N_CORES):
    QB = N // 512
    out = np.empty((B, N, C), dtype=np.float32)
    for i in range(n_cores):
        o = res.results[i]["out"]  # [B, TOKB, C]
        for b in range(B):
            grps = groups_for(b, QB)
            tok0 = 0
            base = 0
            for grp in grps:
                tokg = 512 * len(grp) // n_cores
                lo = base + tokg * i
                out[b, lo:lo + tokg, :] = o[b, tok0:tok0 + tokg]
                tok0 += tokg
                base += 512 * len(grp)
    return out


_NC_CACHE = {}


def _get_program(N, mm_dt=BF16):
    key = (N, str(mm_dt))
    if key not in _NC_CACHE:
        _NC_CACHE[key] = build_program(N=N, mm_dt=mm_dt)
    return _NC_CACHE[key]


def kernel(x, qkv_w, proj_w, proj_b):
    x = np.asarray(x)
    Bx, N, Cx = x.shape
    assert (Bx, Cx) == (B, C), (Bx, Cx)
    nc = _get_program(N)
    in_maps = shard_inputs(x, qkv_w, proj_w, proj_b)
    res = run_bass_kernel_spmd(nc, in_maps, list(range(N_CORES)))
    return assemble_output(res, N)


# revision 45
# speedup vs baseline: 1.3131x; 1.3131x over previous
"""Multi-head attention (B=2, N=2048, C=1024, H=16) on 8 TRN2 NeuronCores.

Sharding: tensor-parallel over heads (2 heads/core) for qkv+attention,
then AllToAll to token-shard the output projection.

v2: one globally software-pipelined schedule built around saturating the
Scalar (activation) engine, which has ~142us of exp work - the hard
bottleneck of this shard layout:
  - attention iterations (sst pair -> exp -> hav pair) start as soon as
    K(0)/Q(0)/V(0..3) exist (~8us), not after the full qkv phase
  - all remaining qkv matmuls (own batch, next batch) and the output
    projections are interleaved into the iteration stream as PE filler,
    budgeted so the PE always has work (keeps its p-state ramped at
    2.4GHz) but never starves ScalarE of sst inputs
  - sst for iteration i+1 is emitted before hav for iteration i so the
    PE never waits on exp latency
  - reciprocal_approx_fast instead of reciprocal (5x) for the softmax
    denominators; normalize chain kept off the critical path
  - per-qb scatter into the AllToAll buffer and per-group gather out of
    it are single strided DMAs (transposed APs) instead of 8-16 small
    serialized DMAs
  - PE warmup matmuls at t=0 and across the final AllToAll wait keep the
    p-state up where there is no real work to overlap
"""

import numpy as np
import ml_dtypes

import concourse.mybir as mybir
import concourse.tile as tile
from concourse import bacc
from concourse.bass_utils import run_bass_kernel_spmd

F32 = mybir.dt.float32
BF16 = mybir.dt.bfloat16
EXP = mybir.ActivationFunctionType.Exp

N_CORES = 8
B = 2
C = 1024
H = 16
D = 64
HPC = H // N_CORES          # heads per core
CH = HPC * D                # channels per core (128)
KT_C = C // 128             # contraction tiles (8)

RELEASE_FIRST = 34          # iters before first collective's proj releases
RELEASE_REST = 22           # iters for later batch-0 collectives
FILL_PER_ITER = 2.2         # filler matmuls per attention iteration
HAV_LAG = 8                 # max iterations hav may trail sst/exp


def groups_for(b, QB):
    g = [list(range(i, min(i + 2, QB))) for i in range(0, QB, 2)]
    if b == B - 1 and len(g) > 1:
        g = g[:-1] + [[q] for q in g[-1]]
    return g


def build_program(N=2048, n_cores=N_CORES, mm_dt=BF16):
    assert N % 512 == 0
    QB = N // 512            # 512-wide query/column blocks per batch
    NK = N // 128            # 128-row key tiles per batch
    scale = float(D) ** -0.5
    TOKB = N // n_cores

    nc = bacc.Bacc("TRN2", target_bir_lowering=False, debug=False,
                   num_devices=n_cores)

    # x host-packed per 512-column block: xR[b, cb, r, k*512+c]
    xR_d = nc.dram_tensor("xR", [B, QB, 128, KT_C * 512], mm_dt,
                          kind="ExternalInput")
    wqT_d = nc.dram_tensor("wqT", [128, KT_C * CH], mm_dt, kind="ExternalInput")
    wkT_d = nc.dram_tensor("wkT", [128, KT_C * CH], mm_dt, kind="ExternalInput")
    wvT_d = nc.dram_tensor("wvT", [128, KT_C * CH], mm_dt, kind="ExternalInput")
    pwT_d = nc.dram_tensor("pwT", [128, KT_C * C], mm_dt, kind="ExternalInput")
    onesb_d = nc.dram_tensor("onesb", [1, 128], mm_dt, kind="ExternalInput")
    pbb_d = nc.dram_tensor("pbb", [C], mm_dt, kind="ExternalInput")
    out_d = nc.dram_tensor("out", [B, TOKB, C], F32, kind="ExternalOutput")

    lp = nc.allow_low_precision("bf16 matmul pipeline")

    with tile.TileContext(nc) as tc:
        with (tc.tile_pool(name="sb", bufs=1) as sb,
              tc.tile_pool(name="ps", bufs=1, space="PSUM") as ps,
              tc.tile_pool(name="dr", bufs=1, space="DRAM") as dr,
              lp):
            # PSUM budget (8 banks): sst 2x2 + hav0 + hav1 + acc 2 (shared
            # by qkv chains, projections, warmup)

            # ---- weights / constants; DMA order: wk then x(0,0) first so
            # the first K chain can start ~4us in ----
            wkt = sb.tile([128, KT_C * CH], mm_dt, tag="wk", bufs=1, name="wk")
            nc.sync.dma_start(wkt[:], wkT_d.ap())

            xt = {}

            def load_x(b, cb, split=False):
                t = sb.tile([128, KT_C * 512], mm_dt, tag="xt", bufs=2 * QB,
                            name=f"x{b}_{cb}")
                if split:
                    # halves (k-major) so K0's first matmuls start earlier
                    half = KT_C * 256
                    nc.sync.dma_start(t[:, 0:half], xR_d.ap()[b, cb][:, 0:half])
                    nc.sync.dma_start(t[:, half:], xR_d.ap()[b, cb][:, half:])
                else:
                    nc.sync.dma_start(t[:], xR_d.ap()[b, cb])
                xt[(b, cb)] = t

            load_x(0, 0, split=True)
            wqt = sb.tile([128, KT_C * CH], mm_dt, tag="wq", bufs=1, name="wq")
            nc.sync.dma_start(wqt[:], wqT_d.ap())
            wvt = sb.tile([128, KT_C * CH], mm_dt, tag="wv", bufs=1, name="wv")
            nc.sync.dma_start(wvt[:], wvT_d.ap())
            onesb = sb.tile([1, 128], mm_dt, tag="onesb", bufs=1)
            nc.sync.dma_start(onesb[:], onesb_d.ap())
            pbb_sb = sb.tile([1, C], mm_dt, tag="pbb", bufs=1)
            nc.sync.dma_start(pbb_sb[:], pbb_d.ap().unsqueeze(0))
            # vau ones-stripes via gpsimd memset (a strided DMA for these
            # costs ~4us of sync-queue time each and delays the x loads)
            vau = [[] for _ in range(B)]
            for vb in range(B):
                for h in range(2):
                    t = sb.tile([128, 65 * NK], mm_dt, tag=f"vau{h}", bufs=2,
                                name=f"vau{vb}_{h}")
                    nc.gpsimd.memset(t[:, 64::65], 1.0)
                    vau[vb].append(t)
            for cb in range(1, QB):
                load_x(0, cb)
            for cb in range(QB):
                load_x(1, cb)
            pwt = sb.tile([128, KT_C * C], mm_dt, tag="pw", bufs=1, name="pw")
            nc.sync.dma_start(pwt[:], pwT_d.ap())

            wk = [wkt[:, CH * k:CH * k + CH] for k in range(KT_C)]
            wq = [wqt[:, CH * k:CH * k + CH] for k in range(KT_C)]
            wv = [wvt[:, CH * k:CH * k + CH] for k in range(KT_C)]
            pw = [pwt[:, C * k:C * k + C] for k in range(KT_C)]

            # ---- PE warmup: ramp the p-state before real work lands ----
            def warmup(n):
                wacc = ps.tile([128, 512], F32, tag="acc", bufs=2,
                               name="warm")
                for i in range(n):
                    nc.tensor.matmul(wacc[:], wkt[:, 0:128], wkt[:, 0:512],
                                     start=(i == 0), stop=(i == n - 1))

            warmup(7)

            # ---- per-batch qkv generators with milestone tracking ----
            kts = [{} for _ in range(B)]
            qts = [{} for _ in range(B)]
            done = [set() for _ in range(B)]

            def qkv_gen(b):
                def xs(cb, k, lo, w):
                    return xt[(b, cb)][:, 512 * k + lo:512 * k + lo + w]

                def kq_chain(which, qb, wlist, store, tag):
                    acc = ps.tile([128, 512], F32, tag="acc", bufs=2,
                                  name=f"{which}acc{b}_{qb}")
                    for k in range(KT_C):
                        nc.tensor.matmul(acc[:], wlist[k], xs(qb, k, 0, 512),
                                         start=(k == 0), stop=(k == KT_C - 1))
                        yield 1
                    tgt = sb.tile([128, 512], mm_dt, tag=tag, bufs=2 * QB + 1,
                                  name=f"{tag}{b}_{qb}")
                    nc.vector.tensor_copy(tgt[:], acc[:])
                    store[qb] = tgt
                    done[b].add((which, qb))

                def k0_chunk(lo, w, kt_t, mark):
                    # first K chain in two column chunks: the first sst tile
                    # only needs keys 0:128, so exp can start ~5us earlier
                    acc = ps.tile([128, w], F32, tag="acc", bufs=2,
                                  name=f"k0acc{b}_{lo}")
                    for k in range(KT_C):
                        nc.tensor.matmul(acc[:], wk[k], xs(0, k, lo, w),
                                         start=(k == 0), stop=(k == KT_C - 1))
                        yield 1
                    nc.vector.tensor_copy(kt_t[:, lo:lo + w], acc[:])
                    done[b].add(mark)

                def v_chain(tt):
                    cb, off = tt // 4, (tt % 4) * 128
                    acc = ps.tile([128, 128], F32, tag="acc", bufs=2,
                                  name=f"vacc{b}_{tt}")
                    for k in range(KT_C):
                        nc.tensor.matmul(acc[:], xs(cb, k, off, 128), wv[k],
                                         start=(k == 0), stop=(k == KT_C - 1))
                        yield 1
                    nc.vector.tensor_copy(
                        vau[b][0][:, 65 * tt:65 * tt + 64], acc[:, 0:64])
                    nc.vector.tensor_copy(
                        vau[b][1][:, 65 * tt:65 * tt + 64], acc[:, 64:128])
                    done[b].add(('V', tt))

                yield from kq_chain('K', 0, wk, kts[b], "kt")
                yield from kq_chain('Q', 0, wq, qts[b], "qt")
                for tt in range(4):
                    yield from v_chain(tt)
                for cb in range(1, QB):
                    yield from kq_chain('K', cb, wk, kts[b], "kt")
                    if cb == 1 and QB > 1:
                        yield from kq_chain('Q', 1, wq, qts[b], "qt")
                    for tt in range(4 * cb, 4 * cb + 4):
                        yield from v_chain(tt)
                for qb in range(2, QB):
                    yield from kq_chain('Q', qb, wq, qts[b], "qt")

            gens = [qkv_gen(b) for b in range(B)]

            units = [0]          # PE matmuls emitted this iteration

            def force(b, key):
                while key not in done[b]:
                    if next(gens[b], None) is None:
                        if key in done[b]:
                            break
                        raise RuntimeError(f"gen {b} dry before {key}")
                    units[0] += 1

            # ---- filler pool (priority order) + proj release queue ----
            def dummy_gen(n):
                # p-state keep-alive when real filler runs dry
                dacc = ps.tile([128, 512], F32, tag="acc", bufs=2,
                               name="dummy")
                for i in range(n):
                    nc.tensor.matmul(dacc[:], wkt[:, 0:128], wkt[:, 0:512],
                                     start=(i == 0), stop=(i == n - 1))
                    yield 1

            fillq = list(gens) + [dummy_gen(60)]
            proj_ripe = []       # (release_at_iter, generator)
            it_counter = [0]

            def fill_pull():
                while fillq:
                    if next(fillq[0], None) is None:
                        fillq.pop(0)
                        continue
                    return True
                return False

            # ---- projection job per (batch, group) ----
            def proj_job(b, a2a_out, tokg, tok0, gi):
                pl = sb.tile([128, KT_C * tokg], mm_dt, tag="pl", bufs=3,
                             name=f"pl{b}_{gi}")
                nc.sync.dma_start(pl.rearrange("p (k t) -> p k t", k=KT_C),
                                  a2a_out.transpose([1, 0, 2]))
                yield 0
                for oh in range(2):
                    os_ = slice(512 * oh, 512 * oh + 512)
                    acc = ps.tile([128, 512], F32, tag="acc", bufs=2,
                                  name=f"pacc{b}_{gi}_{oh}")
                    for k in range(KT_C):
                        nc.tensor.matmul(
                            acc[0:tokg, :], pl[:, tokg * k:tokg * k + tokg],
                            pw[k][:, os_], start=(k == 0), stop=False)
                        yield 1
                    nc.tensor.matmul(acc[0:tokg, :], onesb[0:1, 0:tokg],
                                     pbb_sb[0:1, os_], start=False, stop=True)
                    yield 1
                    osb = sb.tile([128, 512], F32, tag="osb", bufs=2,
                                  name=f"osb{b}_{gi}_{oh}")
                    nc.vector.tensor_copy(osb[0:tokg, :], acc[0:tokg, :])
                    nc.sync.dma_start(
                        out_d.ap()[b, tok0:tok0 + tokg, os_], osb[0:tokg, :])

            # ---- attention building blocks ----
            hav = {}             # h -> current psum tile
            n_coll = [0]

            def emit_sst_exp(b, qb, t):
                force(b, ('K', t // 4))
                force(b, ('Q', qb))
                sst = ps.tile([128, 1024], F32, tag="sst", bufs=2,
                              name=f"sst{b}_{qb}_{t}")
                pt = sb.tile([128, 1024], mm_dt, tag="pt", bufs=14,
                             name=f"pt{b}_{qb}_{t}")
                kb, ko = t // 4, t % 4
                ks = slice(128 * ko, 128 * ko + 128)
                for h in range(2):
                    hs = slice(64 * h, 64 * h + 64)
                    nc.tensor.matmul(sst[:, 512 * h:512 * h + 512],
                                     kts[b][kb][hs, ks], qts[b][qb][hs, :],
                                     start=True, stop=True)
                units[0] += 1      # pair runs concurrently: one PE slot
                nc.scalar.activation(pt[:], sst[:], EXP, scale=scale)
                return pt

            grp_state = {}       # (b, gi) -> dict with a2a_in etc.

            def emit_hav(b, qb, t, pt):
                force(b, ('V', t))
                if t == 0:
                    for h in range(2):
                        hav[h] = ps.tile([65, 512], F32, tag=f"hav{h}",
                                         bufs=1, name=f"hav{b}_{qb}_{h}")
                for h in range(2):
                    nc.tensor.matmul(hav[h][:],
                                     vau[b][h][:, 65 * t:65 * t + 65],
                                     pt[:, 512 * h:512 * h + 512],
                                     start=(t == 0), stop=(t == NK - 1))
                units[0] += 2
                if t == NK - 1:
                    finish_qb(b, qb)

            def finish_qb(b, qb):
                grps = groups_for(b, QB)
                gi = next(i for i, g in enumerate(grps) if qb in g)
                grp = grps[gi]
                p = grp.index(qb)
                tokg = 512 * len(grp) // n_cores
                npr = 512 // tokg
                st = grp_state.setdefault((b, gi), {})
                if "a2a_in" not in st:
                    st["a2a_in"] = dr.tile([n_cores, CH, tokg], mm_dt,
                                           tag="a2ain", bufs=5,
                                           name=f"a2ain{b}_{gi}")
                # normalize: h/denom with approx reciprocal, then one
                # strided scatter DMA into the group's AllToAll buffer
                ht = sb.tile([128, 512], mm_dt, tag="ht", bufs=2,
                             name=f"ht{b}_{qb}")
                for h in range(2):
                    # one copy frees the hav PSUM bank for the next qb as
                    # early as possible (the next block's matmuls wait on it)
                    hcp = sb.tile([65, 512], F32, tag=f"hcp{h}", bufs=2,
                                  name=f"hcp{b}_{qb}_{h}")
                    nc.vector.tensor_copy(hcp[:], hav[h][:])
                    # denominator row to a partition-0 tile: the custom-DVE
                    # approx reciprocal mishandles base_partition != 0
                    dnm = sb.tile([1, 512], F32, tag=f"dnm{h}", bufs=2,
                                  name=f"dnm{b}_{qb}_{h}")
                    nc.vector.tensor_copy(dnm[:], hcp[64:65, :])
                    nrr = sb.tile([1, 512], F32, tag=f"nrr{h}", bufs=2,
                                  name=f"nrr{b}_{qb}_{h}")
                    nc.vector.reciprocal_approx_fast(nrr[:], dnm[:])
                    bcs = sb.tile([64, 512], F32, tag=f"bcs{h}", bufs=2,
                                  name=f"bcs{b}_{qb}_{h}")
                    nc.gpsimd.partition_broadcast(bcs[:], nrr[:])
                    nc.vector.tensor_mul(ht[64 * h:64 * h + 64, :],
                                         hcp[0:64, :], bcs[:])
                a2a_in = st["a2a_in"]
                nc.sync.dma_start(
                    a2a_in[p * npr:(p + 1) * npr].transpose([1, 0, 2]),
                    ht.rearrange("p (s t) -> p s t", s=npr))
                if p == len(grp) - 1:
                    a2a_out = dr.tile([n_cores, CH, tokg], mm_dt,
                                      tag="a2aout", bufs=5,
                                      name=f"a2aout{b}_{gi}")
                    nc.gpsimd.collective_compute(
                        "AllToAll", mybir.AluOpType.bypass,
                        replica_groups=[list(range(n_cores))],
                        ins=[a2a_in.opt()], outs=[a2a_out.opt()])
                    tok0 = sum(512 * len(g) // n_cores for g in grps[:gi])
                    # last batch's projs run in the drain, where they double
                    # as PE filler across the final collectives' latency
                    if b == B - 1:
                        delay = 10 ** 9
                    elif n_coll[0] == 0:
                        delay = RELEASE_FIRST
                    else:
                        delay = RELEASE_REST
                    n_coll[0] += 1
                    proj_ripe.append((it_counter[0] + delay,
                                      proj_job(b, a2a_out, tokg, tok0, gi)))

            # ---- the global iteration stream ----
            iters = [(b, qb, t) for b in range(B) for qb in range(QB)
                     for t in range(NK)]
            pending = []
            credit = 0.0
            for (b, qb, t) in iters:
                units[0] = 0
                pt = emit_sst_exp(b, qb, t)
                pending.append((b, qb, t, pt))
                # emit hav once its V tile exists, or when the lag cap hits;
                # deferring spreads the V-chain crunch past the first qb
                while pending and (len(pending) > HAV_LAG
                                   or ('V', pending[0][2]) in done[pending[0][0]]):
                    emit_hav(*pending.pop(0))
                # release ripe proj jobs into the filler pool, ahead of the
                # dummy keep-alive generator at the tail of the queue
                for ent in list(proj_ripe):
                    if ent[0] <= it_counter[0]:
                        fillq.insert(max(0, len(fillq) - 1), ent[1])
                        proj_ripe.remove(ent)
                # budget: attention fixed cost ~3 slots vs exp cadence ~5.2
                credit += FILL_PER_ITER + 3.0 - units[0]
                credit = max(-6.0, min(8.0, credit))
                while credit >= 1.0 and fill_pull():
                    credit -= 1.0
                it_counter[0] += 1

            # ---- drain: final havs + chain, remaining filler, tail projs ----
            # prime the pl loads of already-collected groups first, so they
            # are not head-of-line blocked behind the final scatter on the
            # sync queue (the last group's own pl must NOT be primed here:
            # its AllToAll depends on the scatter emitted below)
            proj_ripe.sort(key=lambda e: e[0])
            primed = [g for _, g in proj_ripe]
            proj_ripe.clear()
            for g in primed:
                next(g, None)
            while pending:
                emit_hav(*pending.pop(0))
            while fill_pull():
                pass
            jobs = primed + [g for _, g in proj_ripe]
            proj_ripe.clear()
            for g in jobs[:-1]:
                while next(g, None) is not None:
                    pass
            # keep the PE ramped across the last AllToAll's latency
            warmup(40)
            for g in jobs[-1:]:
                while next(g, None) is not None:
                    pass

    nc.compile()
    return nc


def shard_inputs(x, qkv_w, proj_w, proj_b, n_cores=N_CORES, mm_dt=BF16):
    """Host-side sharding: pre-pack activations/weights, slice heads."""
    npdt = ml_dtypes.bfloat16 if mm_dt == BF16 else np.float32
    x = np.asarray(x)
    Bx, N, Cx = x.shape
    QB = N // 512
    xT = np.ascontiguousarray(np.transpose(x, (0, 2, 1))).astype(npdt)
    # xR[b, cb, r, k*512+c] = xT[b, 128k+r, 512cb+c]
    xR = np.ascontiguousarray(
        xT.reshape(Bx, KT_C, 128, QB, 512).transpose(0, 3, 2, 1, 4)
        .reshape(Bx, QB, 128, KT_C * 512))
    qkv_w = np.asarray(qkv_w)

    def pack(wT):  # [C, cols] -> [128, (C//128)*cols], k-tiles side by side
        cdim, cols = wT.shape
        return np.ascontiguousarray(
            wT.reshape(cdim // 128, 128, cols).transpose(1, 0, 2)
            .reshape(128, -1)).astype(npdt)

    pwT = pack(np.asarray(proj_w).T)
    pb = np.ascontiguousarray(np.asarray(proj_b)).astype(np.float32)
    in_maps = []
    for i in range(n_cores):
        cs = slice(CH * i, CH * i + CH)
        in_maps.append({
            "xR": xR,
            "wqT": pack(qkv_w[cs, :].T),
            "wkT": pack(qkv_w[C:][cs, :].T),
            "wvT": pack(qkv_w[2 * C:][cs, :].T),
            "pwT": pwT,
            "onesb": np.ones((1, 128), dtype=npdt),
            "pbb": pb.astype(npdt),
        })
    return in_maps


def assemble_output(res, N, n_cores=N_CORES):
    QB = N // 512
    out = np.empty((B, N, C), dtype=np.float32)
    for i in range(n_cores):
        o = res.results[i]["out"]  # [B, TOKB, C]
        for b in range(B):
            grps = groups_for(b, QB)
            tok0 = 0
            base = 0
            for grp in grps:
                tokg = 512 * len(grp) // n_cores
                lo = base + tokg * i
                out[b, lo:lo + tokg, :] = o[b, tok0:tok0 + tokg]
                tok0 += tokg
                base += 512 * len(grp)
    return out


_NC_CACHE = {}


def _get_program(N, mm_dt=BF16):
    key = (N, str(mm_dt))
    if key not in _NC_CACHE:
        _NC_CACHE[key] = build_program(N=N, mm_dt=mm_dt)
    return _NC_CACHE[key]


def kernel(x, qkv_w, proj_w, proj_b):
    x = np.asarray(x)
    Bx, N, Cx = x.shape
    assert (Bx, Cx) == (B, C), (Bx, Cx)
    nc = _get_program(N)
    in_maps = shard_inputs(x, qkv_w, proj_w, proj_b)
    res = run_bass_kernel_spmd(nc, in_maps, list(range(N_CORES)))
    return assemble_output(res, N)
